# revision 41
# baseline (speedup 1.0000x reference)
"""Trainium2 Bass kernel for nn_KuramotoHyperUniversal.

Data-parallel over batch across 8 NeuronCores (64 rows/core); weights
replicated. The (B,D,D) pairwise term is computed via the identity
  sum_j sin(y_j - y_i) A[i,j] = cos(y_i)*(A@sin(y))_i - sin(y_i)*(A@cos(y))_i

v2 design — memory-regime optimized:
- All weights cast to bf16 on the host and pre-chunked into m-major SBUF
  mega layouts [128, 12 blocks of 12*128 cols (+ m-tail block)], so each
  layer streams from HBM with 3-4 large contiguous DMAs in consumption
  order (~16.6 MB/core total vs 33 MB fp32).
- Fully transposed dataflow: activations live feature-on-partition,
  batch-on-free ([128, 64] chunks). Each matmul uses the weight chunk as
  the stationary operand (M=128, full PE width) and the activation chunk
  as the moving operand, producing the next layer's input layout
  directly — zero on-device transposes.
- Biases and the constant t-column are folded in as K=1 matmuls against
  a broadcast row; tanh runs on ACT over [128,64] PSUM chunks.
- Output is packed transposed into one [128, 5*64] tile (4 forceT chunks
  + f1 row); the host reassembles [512, 513].
"""

import numpy as np
import ml_dtypes
from contextlib import ExitStack

import concourse.bass as bass
import concourse.mybir as mybir
import concourse.tile as tile
from concourse.vector_clock import ScopedClock, VectorClock
from concourse.bass_utils import run_bass_kernel_spmd

BF16 = ml_dtypes.bfloat16
DIM = 512
BATCH = 512
NCORES = 8
BS = BATCH // NCORES  # 64
H = 2 + 3 * DIM  # 1538
F32 = mybir.dt.float32
BF = mybir.dt.bfloat16
PI_HALF = float(np.pi / 2.0)


def _split_drain_and_barrier(self, tick_clock, wait_clock):
    # Walrus in this container rejects >2 sync waits on one CTRL (drain)
    # instruction; emit one single-wait NOP per outstanding proc instead.
    gc = tick_clock.global_clock
    ticks = list(gc)
    for p, t in enumerate(ticks):
        if t > 0:
            v = [0] * len(ticks)
            v[p] = t
            nop = self.nc.sync.nop(nofuse=True, hint=f"drain_wait_{p}")
            wait_clock.add_sem_waits(nop.ins, ScopedClock({None: VectorClock(v)}))
    self.nc.sync.drain()
    self.nc.all_engine_barrier()
    popped = self.nc._tile_sem_poison_stack.pop()
    assert popped is self._sem_poison
    self.nc.clear_and_free_semaphores(list(self.sems.allocated().values()))
    self.nc.all_engine_barrier()


tile.TileContext._drain_and_barrier = _split_drain_and_barrier

_MAX_WAITS = 1


def _split_waits(nc, limit=_MAX_WAITS):
    """Walrus rejects instructions carrying more than `limit` sync waits;
    move the excess onto same-engine NOPs inserted just before."""
    import bass_rust

    n = 0
    for f in nc.m.functions:
        for bb in f.blocks:
            out = []
            for inst in bb.instructions:
                si = inst.sync_info
                if si is not None and si.on_wait and len(si.on_wait) > limit:
                    waits = list(si.on_wait)
                    extra, keep = waits[:-limit], waits[-limit:]
                    for i in range(0, len(extra), limit):
                        nop = mybir.InstNoOp(name=f"I-wsplit-{n}", engine=inst.engine)
                        n += 1
                        nop.sync_info = bass_rust.SyncInfo(
                            on_wait=extra[i : i + limit], on_update=[]
                        )
                        out.append(nop)
                    inst.sync_info = bass_rust.SyncInfo(
                        on_wait=keep, on_update=list(si.on_update)
                    )
                out.append(inst)
            bb.instructions = out


# m-chunking of the hidden dim: 12 chunks of 128 + tail of 2
M_CHUNKS = [(m * 128, 128) for m in range(12)] + [(1536, 2)]
M3_CHUNKS = [(m * 128, 128) for m in range(4)]  # out dim 512


def _build(reps=1, loop=None):
    nc = bass.Bass()
    AF = mybir.ActivationFunctionType

    # ypk: yT chunks [0:256] | fqTf chunks [256:512] | t (row 0, col 512)
    ypk_p = nc.declare_dram_parameter("ypk", [128, 8 * BS + 1], F32, isOutput=False)
    # apk: ATm [0:2048] | fqT bf16 chunks [2048:2304]
    apk_p = nc.declare_dram_parameter("apk", [128, 4 * DIM + 4 * BS], BF, isOutput=False)
    W0m_p = nc.declare_dram_parameter("W0m", [128, 12 * H], BF, isOutput=False)
    W1m_p = nc.declare_dram_parameter("W1m", [128, 12 * H], BF, isOutput=False)
    W1t_p = nc.declare_dram_parameter("W1t", [2, H], BF, isOutput=False)
    W2m_p = nc.declare_dram_parameter("W2m", [128, 12 * H], BF, isOutput=False)
    W2t_p = nc.declare_dram_parameter("W2t", [2, H], BF, isOutput=False)
    W3m_p = nc.declare_dram_parameter("W3m", [128, 12 * DIM], BF, isOutput=False)
    W3t_p = nc.declare_dram_parameter("W3t", [2, DIM], BF, isOutput=False)
    # b0 | b1 | b2 | w0row (4x H) | b3 (DIM), packed into one row
    SPK = 4 * H + DIM
    spk_p = nc.declare_dram_parameter("spk", [1, SPK], BF, isOutput=False)
    out_p = nc.declare_dram_parameter("out", [128, 5 * BS], F32, isOutput=True)

    with ExitStack() as ctx:
        tc = ctx.enter_context(tile.TileContext(nc))
        const = ctx.enter_context(tc.tile_pool(name="const", bufs=1))
        wpool = ctx.enter_context(tc.tile_pool(name="wpool", bufs=1))
        io = ctx.enter_context(tc.tile_pool(name="io", bufs=1))
        hp = ctx.enter_context(tc.tile_pool(name="hp", bufs=26))
        ps = ctx.enter_context(tc.tile_pool(name="ps", bufs=8, space="PSUM"))

        ones_row = const.tile([1, BS], BF, tag="ones_row")
        nc.vector.memset(ones_row[:], 1.0)
        ones128 = const.tile([128, 1], BF, tag="ones128")
        nc.vector.memset(ones128[:], 1.0)
        ones_f = const.tile([1, BS], F32, tag="ones_f")
        nc.vector.memset(ones_f[:], 1.0)
        pih = const.tile([128, 1], F32, tag="pih")
        nc.vector.memset(pih[:], PI_HALF)

        def _emit(rep):
            # ---- input DMAs, in consumption order ----
            ypk = io.tile([128, 8 * BS + 1], F32, tag="ypk")
            nc.sync.dma_start(out=ypk[:], in_=ypk_p[:])
            yT_j = lambda j: ypk[:, j * BS : (j + 1) * BS]
            fqTf_j = lambda j: ypk[:, (4 + j) * BS : (5 + j) * BS]
            t_sb = ypk[0:1, 8 * BS : 8 * BS + 1]
            spk = io.tile([1, SPK], BF, tag="spk")
            nc.sync.dma_start(out=spk[:], in_=spk_p[:])
            apk = wpool.tile([128, 4 * DIM + 4 * BS], BF, tag="apk")
            nc.sync.dma_start(out=apk[:], in_=apk_p[:])
            ATm_ji = lambda j, i: apk[:, j * DIM + i * 128 : j * DIM + (i + 1) * 128]
            fqT_j = lambda j: apk[:, 4 * DIM + j * BS : 4 * DIM + (j + 1) * BS]
            brow = [spk[:, l * H : l * H + n] for l, n in zip(range(3), [H, H, H])]
            brow.append(spk[:, 4 * H : 4 * H + DIM])
            w0row = spk[:, 3 * H : 4 * H]

            # ---- big weight DMAs: per-layer tail first, then mega chunks.
            # Megas are m-major (12 blocks of 12*128 cols + optional m-tail
            # block of 24), chunked on block boundaries so each m-group only
            # depends on its own chunk's arrival.
            Wm, Wt = {}, {}
            for nm, p, bounds, tnm, tp, tn in (
                ("W0m", W0m_p, (0, 6144, 12288, 18456), None, None, 0),
                ("W1m", W1m_p, (0, 6144, 12288, 18456), "W1t", W1t_p, H),
                ("W2m", W2m_p, (0, 6144, 12288, 18456), "W2t", W2t_p, H),
                ("W3m", W3m_p, (0, 1536, 3072, 4608, 6144), "W3t", W3t_p, DIM),
            ):
                if tnm is not None:
                    wtt = wpool.tile([2, tn], BF, tag=tnm, name=tnm)
                    nc.sync.dma_start(out=wtt[:], in_=tp[:])
                    Wt[tnm] = wtt
                wt = wpool.tile([128, bounds[-1]], BF, tag=nm, name=nm)
                for a, b in zip(bounds[:-1], bounds[1:]):
                    nc.sync.dma_start(out=wt[:, a:b], in_=p[:, a:b])
                Wm[nm] = wt

            # ---- t row: tm1row[1,64] = (t - 1) broadcast, bf16 ----
            tm1 = io.tile([1, 1], F32, tag="tm1")
            nc.vector.tensor_scalar_add(tm1[:], t_sb[:], -1.0)
            tm1row_f = io.tile([1, BS], F32, tag="tm1row_f")
            nc.vector.tensor_scalar_mul(tm1row_f[:], ones_f[:], tm1[:])
            tm1row = io.tile([1, BS], BF, tag="tm1row")
            nc.vector.tensor_copy(tm1row[:], tm1row_f[:])

            # ---- xC/xS: cos/sin of phases, feature-on-partition bf16 ----
            xC, xS = [], []
            for j in range(4):
                c = io.tile([128, BS], BF, tag=f"xC{j}", name=f"xC{j}")
                nc.scalar.activation(c[:], yT_j(j), AF.Sin, bias=pih[:])
                xC.append(c)
                s = io.tile([128, BS], BF, tag=f"xS{j}", name=f"xS{j}")
                nc.scalar.activation(s[:], yT_j(j), AF.Sin)
                xS.append(s)

            # ---- trig matmuls: AST/ACT [feature, batch], fsT combine ----
            psAS = ps.tile([128, 4 * BS], F32, tag="pbank", name="psAS")
            psAC = ps.tile([128, 4 * BS], F32, tag="pbank", name="psAC")
            for j in range(4):
                for pt, x in ((psAS, xS), (psAC, xC)):
                    for i in range(4):
                        nc.tensor.matmul(
                            pt[:, i * BS : (i + 1) * BS],
                            ATm_ji(j, i),
                            x[j][:],
                            start=(j == 0),
                            stop=(j == 3),
                        )
            fsT = []
            for i in range(4):
                f = io.tile([128, BS], F32, tag=f"fsT{i}", name=f"fsT{i}")
                nc.vector.tensor_mul(f[:], xC[i][:], psAS[:, i * BS : (i + 1) * BS])
                tmp = io.tile([128, BS], F32, tag="fstmp", bufs=2)
                nc.vector.tensor_mul(tmp[:], xS[i][:], psAC[:, i * BS : (i + 1) * BS])
                nc.vector.tensor_sub(f[:], f[:], tmp[:])
                nc.vector.tensor_scalar_mul(f[:], f[:], 1.0 / DIM)
                fsT.append(f)

            # ---- MLP layers, transposed dataflow ----
            def layer(Wm_t, Wtail_t, ktail_rhs, brow_t, rhs, mchunks, extra_k=None):
                """hT[m] = act(sum_k Wchunk[k,m]^T @ rhs[k] + b[m]).
                Wm_t is the m-major mega: block m at m*1536, m-tail block (if
                any) at 12*1536 with k-stride 2."""
                outs = []
                for moff, mw in mchunks:
                    p = ps.tile([128, BS], F32, tag="pbank", name=f"ps_{moff}")
                    pa = p[0:mw, :]
                    for k in range(12):
                        off = (
                            (moff // 128) * 1536 + k * 128
                            if mw == 128
                            else 12 * 1536 + k * 2
                        )
                        nc.tensor.matmul(
                            pa,
                            Wm_t[:, off : off + mw],
                            rhs[k],
                            start=(k == 0),
                            stop=False,
                        )
                    if Wtail_t is not None:
                        nc.tensor.matmul(
                            pa,
                            Wtail_t[:, moff : moff + mw],
                            ktail_rhs,
                            start=False,
                            stop=False,
                        )
                    if extra_k is not None:
                        xrow, xrhs = extra_k
                        nc.tensor.matmul(
                            pa,
                            xrow[:, moff : moff + mw],
                            xrhs[:],
                            start=False,
                            stop=False,
                        )
                    nc.tensor.matmul(
                        pa,
                        brow_t[:, moff : moff + mw],
                        ones_row[:],
                        start=False,
                        stop=True,
                    )
                    outs.append((p, pa, moff, mw))
                return outs

            def act_chunks(pre, fn):
                outs = []
                for p, pa, moff, mw in pre:
                    h = hp.tile([128, BS], BF, tag="h", name=f"h_{moff}")
                    nc.scalar.activation(h[0:mw, :], pa, fn)
                    outs.append(h)
                return outs

            l0_rhs = (
                [c[:] for c in xC]
                + [s[:] for s in xS]
                + [fqT_j(j) for j in range(4)]
            )
            l0 = layer(
                Wm["W0m"], None, None, brow[0],
                l0_rhs, M_CHUNKS, extra_k=(w0row, tm1row),
            )
            h = act_chunks(l0, AF.Tanh)
            for l in (1, 2):
                pre = layer(
                    Wm[f"W{l}m"], Wt[f"W{l}t"], h[12][0:2, :], brow[l],
                    [hh[:] for hh in h[:12]], M_CHUNKS,
                )
                h = act_chunks(pre, AF.Tanh)
            l3 = layer(
                Wm["W3m"], Wt["W3t"], h[12][0:2, :], brow[3],
                [hh[:] for hh in h[:12]], M3_CHUNKS,
            )

            # ---- outputs: force chunks (one big DMA, fires as soon as the
            # last add lands) + f1 (tiny DMA, overlaps the big one) ----
            psf1 = ps.tile([1, BS], F32, tag="pbank", name="psf1")
            fmall = io.tile([128, 5 * BS], F32, tag="fmall")
            for mi, (p, pa, moff, mw) in enumerate(l3):
                fs_ = fmall[:, mi * BS : (mi + 1) * BS]
                nc.vector.tensor_mul(fs_, pa, fsT[mi][:])
                nc.vector.tensor_add(fs_, fs_, fqTf_j(mi))
                sq = io.tile([128, BS], BF, tag="sq", name=f"sq{mi}", bufs=4)
                nc.scalar.activation(sq[:], pa, AF.Square)
                nc.tensor.matmul(
                    psf1[:], ones128[:], sq[:], start=(mi == 0), stop=(mi == 3)
                )
            nc.sync.dma_start(out=out_p[:, 0 : 4 * BS], in_=fmall[:, 0 : 4 * BS])
            nc.vector.tensor_copy(fmall[0:1, 4 * BS : 5 * BS], psf1[:])
            nc.sync.dma_start(
                out=out_p[0:1, 4 * BS : 5 * BS], in_=fmall[0:1, 4 * BS : 5 * BS]
            )

        if loop is not None:
            with tc.For_i(0, loop, 1):
                for _rep in range(reps):
                    _emit(_rep)
        else:
            for _rep in range(reps):
                _emit(_rep)

    _split_waits(nc)
    return nc


def _prep_shared(inputs):
    """Host-side: cast weights to bf16 and pre-chunk into SBUF mega layouts.
    Weight megas are m-major: 12 blocks [12*128 cols] (block m holds cols
    m*128:(m+1)*128 of W for all 12 k-chunks), then an optional m-tail
    block [12*2 cols] for out-dim 1538."""
    f32 = lambda k: np.asarray(inputs[k], np.float32)

    def mega_mm(Wk, out_w):
        # Wk: [1536, out_w] (k rows only)
        blocks = [
            np.ascontiguousarray(Wk[:, m * 128 : (m + 1) * 128])
            .reshape(12, 128, 128)
            .transpose(1, 0, 2)
            .reshape(128, 12 * 128)
            for m in range(out_w // 128)
        ]
        if out_w % 128:
            blocks.append(
                np.ascontiguousarray(Wk[:, (out_w // 128) * 128 :])
                .reshape(12, 128, out_w % 128)
                .transpose(1, 0, 2)
                .reshape(128, 12 * (out_w % 128))
            )
        return np.ascontiguousarray(np.concatenate(blocks, axis=1)).astype(BF16)

    def mega_km(W, ncol):
        n = W.shape[0] // 128 * 128
        return np.ascontiguousarray(
            W[:n].reshape(-1, 128, ncol).transpose(1, 0, 2).reshape(128, -1)
        ).astype(BF16)

    W0 = f32("W0")
    w0x = np.concatenate([W0[0:1024], W0[1025:1537]], axis=0)  # drop t row
    A = f32("A")
    AT = np.ascontiguousarray(A.T)
    # spk row: b0 | b1 | b2 | w0row | b3
    spk = np.concatenate(
        [f32("b0"), f32("b1"), f32("b2"), W0[1024], f32("b3")]
    ).reshape(1, -1)
    shared = {
        "W0m": mega_mm(w0x, H),
        "W1m": mega_mm(f32("W1")[:1536], H),
        "W1t": np.ascontiguousarray(f32("W1")[1536:1538]).astype(BF16),
        "W2m": mega_mm(f32("W2")[:1536], H),
        "W2t": np.ascontiguousarray(f32("W2")[1536:1538]).astype(BF16),
        "W3m": mega_mm(f32("W3")[:1536], DIM),
        "W3t": np.ascontiguousarray(f32("W3")[1536:1538]).astype(BF16),
        "spk": spk.astype(BF16),
    }
    return shared, mega_km(AT, DIM), f32("t")


def _make_in_maps(inputs):
    shared, ATm, t = _prep_shared(inputs)
    y = np.asarray(inputs["y"], np.float32)
    fq = np.asarray(inputs["freqs"], np.float32)
    maps = []
    for c in range(NCORES):
        m = dict(shared)
        yT = np.ascontiguousarray(y[c * BS : (c + 1) * BS, 0:DIM].T)  # [512, 64]
        yTc = yT.reshape(4, 128, BS).transpose(1, 0, 2).reshape(128, 4 * BS)
        fqT = np.ascontiguousarray(fq[c * BS : (c + 1) * BS].T)
        fqTc = fqT.reshape(4, 128, BS).transpose(1, 0, 2).reshape(128, 4 * BS)
        ypk = np.zeros((128, 8 * BS + 1), np.float32)
        ypk[:, 0 : 4 * BS] = yTc
        ypk[:, 4 * BS : 8 * BS] = fqTc
        ypk[0, 8 * BS] = t[0]
        m["ypk"] = ypk
        apk = np.concatenate([ATm, fqTc.astype(BF16)], axis=1)
        m["apk"] = np.ascontiguousarray(apk)
        maps.append(m)
    return maps


def _assemble(out_full):
    """out_full: [NCORES, 128, 5*BS] -> [BATCH, DIM+1].
    Core layout: cols mi*BS:(mi+1)*BS hold forceT chunk mi ([feature 128,
    batch BS]); row 0 of cols 4*BS:5*BS holds f1."""
    outs = []
    for c in range(NCORES):
        O = out_full[c]
        force = (
            O[:, : 4 * BS].reshape(128, 4, BS).transpose(1, 0, 2).reshape(DIM, BS).T
        )  # [BS, DIM]
        f1 = O[0, 4 * BS : 5 * BS]  # [BS]
        outs.append(np.concatenate([force, f1[:, None]], axis=1))
    return np.ascontiguousarray(np.concatenate(outs, axis=0)).astype(np.float32)


_NC_CACHE = {}


def kernel(**inputs):
    key = "nc"
    if key not in _NC_CACHE:
        _NC_CACHE[key] = _build()
    nc = _NC_CACHE[key]

    in_maps = _make_in_maps(inputs)
    res = run_bass_kernel_spmd(nc, in_maps, core_ids=list(range(NCORES)))
    out_full = np.stack([res.results[i]["out"] for i in range(NCORES)], axis=0)
    return _assemble(out_full)


# revision 43
# speedup vs baseline: 1133.5631x; 1133.5631x over previous
"""Trainium2 Bass kernel for nn_KuramotoHyperUniversal.

Data-parallel over batch across 8 NeuronCores (64 rows/core); weights
replicated. The (B,D,D) pairwise term is computed via the identity
  sum_j sin(y_j - y_i) A[i,j] = cos(y_i)*(A@sin(y))_i - sin(y_i)*(A@cos(y))_i

v2 design — memory-regime optimized:
- All weights cast to bf16 on the host and pre-chunked into m-major SBUF
  mega layouts [128, 12 blocks of 12*128 cols (+ m-tail block)], so each
  layer streams from HBM with 3-4 large contiguous DMAs in consumption
  order (~16.6 MB/core total vs 33 MB fp32).
- Fully transposed dataflow: activations live feature-on-partition,
  batch-on-free ([128, 64] chunks). Each matmul uses the weight chunk as
  the stationary operand (M=128, full PE width) and the activation chunk
  as the moving operand, producing the next layer's input layout
  directly — zero on-device transposes.
- Biases and the constant t-column are folded in as K=1 matmuls against
  a broadcast row; tanh runs on ACT over [128,64] PSUM chunks.
- Output is packed transposed into one [128, 5*64] tile (4 forceT chunks
  + f1 row); the host reassembles [512, 513].
"""

import numpy as np
import ml_dtypes
from contextlib import ExitStack

import concourse.bass as bass
import concourse.mybir as mybir
import concourse.tile as tile
from concourse.vector_clock import ScopedClock, VectorClock
from concourse.bass_utils import run_bass_kernel_spmd

BF16 = ml_dtypes.bfloat16
DIM = 512
BATCH = 512
NCORES = 8
BS = BATCH // NCORES  # 64
H = 2 + 3 * DIM  # 1538
F32 = mybir.dt.float32
BF = mybir.dt.bfloat16
PI_HALF = float(np.pi / 2.0)


def _split_drain_and_barrier(self, tick_clock, wait_clock):
    # Walrus in this container rejects >2 sync waits on one CTRL (drain)
    # instruction; emit one single-wait NOP per outstanding proc instead.
    gc = tick_clock.global_clock
    ticks = list(gc)
    for p, t in enumerate(ticks):
        if t > 0:
            v = [0] * len(ticks)
            v[p] = t
            nop = self.nc.sync.nop(nofuse=True, hint=f"drain_wait_{p}")
            wait_clock.add_sem_waits(nop.ins, ScopedClock({None: VectorClock(v)}))
    self.nc.sync.drain()
    self.nc.all_engine_barrier()
    popped = self.nc._tile_sem_poison_stack.pop()
    assert popped is self._sem_poison
    self.nc.clear_and_free_semaphores(list(self.sems.allocated().values()))
    self.nc.all_engine_barrier()


tile.TileContext._drain_and_barrier = _split_drain_and_barrier

_MAX_WAITS = 1


def _split_waits(nc, limit=_MAX_WAITS):
    """Walrus rejects instructions carrying more than `limit` sync waits;
    move the excess onto same-engine NOPs inserted just before."""
    import bass_rust

    n = 0
    for f in nc.m.functions:
        for bb in f.blocks:
            out = []
            for inst in bb.instructions:
                si = inst.sync_info
                if si is not None and si.on_wait and len(si.on_wait) > limit:
                    waits = list(si.on_wait)
                    extra, keep = waits[:-limit], waits[-limit:]
                    for i in range(0, len(extra), limit):
                        nop = mybir.InstNoOp(name=f"I-wsplit-{n}", engine=inst.engine)
                        n += 1
                        nop.sync_info = bass_rust.SyncInfo(
                            on_wait=extra[i : i + limit], on_update=[]
                        )
                        out.append(nop)
                    inst.sync_info = bass_rust.SyncInfo(
                        on_wait=keep, on_update=list(si.on_update)
                    )
                out.append(inst)
            bb.instructions = out


# m-chunking of the hidden dim: 12 chunks of 128 + tail of 2
M_CHUNKS = [(m * 128, 128) for m in range(12)] + [(1536, 2)]
M3_CHUNKS = [(m * 128, 128) for m in range(4)]  # out dim 512


def _build(reps=1, loop=None):
    nc = bass.Bass()
    AF = mybir.ActivationFunctionType

    # ypk: yT chunks [0:256] | fqTf chunks [256:512] | t (row 0, col 512)
    ypk_p = nc.declare_dram_parameter("ypk", [128, 8 * BS + 1], F32, isOutput=False)
    # apk: ATm [0:2048] | fqT bf16 chunks [2048:2304]
    apk_p = nc.declare_dram_parameter("apk", [128, 4 * DIM + 4 * BS], BF, isOutput=False)
    W0m_p = nc.declare_dram_parameter("W0m", [128, 12 * H], BF, isOutput=False)
    W1m_p = nc.declare_dram_parameter("W1m", [128, 12 * H], BF, isOutput=False)
    W1t_p = nc.declare_dram_parameter("W1t", [2, H], BF, isOutput=False)
    W2m_p = nc.declare_dram_parameter("W2m", [128, 12 * H], BF, isOutput=False)
    W2t_p = nc.declare_dram_parameter("W2t", [2, H], BF, isOutput=False)
    W3m_p = nc.declare_dram_parameter("W3m", [128, 12 * DIM], BF, isOutput=False)
    W3t_p = nc.declare_dram_parameter("W3t", [2, DIM], BF, isOutput=False)
    # b0 | b1 | b2 | w0row (4x H) | b3 (DIM), packed into one row
    SPK = 4 * H + DIM
    spk_p = nc.declare_dram_parameter("spk", [1, SPK], BF, isOutput=False)
    out_p = nc.declare_dram_parameter("out", [128, 5 * BS], F32, isOutput=True)

    with ExitStack() as ctx:
        tc = ctx.enter_context(tile.TileContext(nc))
        const = ctx.enter_context(tc.tile_pool(name="const", bufs=1))
        wpool = ctx.enter_context(tc.tile_pool(name="wpool", bufs=1))
        io = ctx.enter_context(tc.tile_pool(name="io", bufs=1))
        hp = ctx.enter_context(tc.tile_pool(name="hp", bufs=26))
        ps = ctx.enter_context(tc.tile_pool(name="ps", bufs=8, space="PSUM"))

        ones_row = const.tile([1, BS], BF, tag="ones_row")
        nc.vector.memset(ones_row[:], 1.0)
        ones128 = const.tile([128, 1], BF, tag="ones128")
        nc.vector.memset(ones128[:], 1.0)
        ones_f = const.tile([1, BS], F32, tag="ones_f")
        nc.vector.memset(ones_f[:], 1.0)
        pih = const.tile([128, 1], F32, tag="pih")
        nc.vector.memset(pih[:], PI_HALF)

        def _emit(rep):
            # ---- input DMAs, in consumption order ----
            ypk = io.tile([128, 8 * BS + 1], F32, tag="ypk")
            nc.sync.dma_start(out=ypk[:], in_=ypk_p[:])
            yT_j = lambda j: ypk[:, j * BS : (j + 1) * BS]
            fqTf_j = lambda j: ypk[:, (4 + j) * BS : (5 + j) * BS]
            t_sb = ypk[0:1, 8 * BS : 8 * BS + 1]
            spk = io.tile([1, SPK], BF, tag="spk")
            nc.sync.dma_start(out=spk[:], in_=spk_p[:])
            apk = wpool.tile([128, 4 * DIM + 4 * BS], BF, tag="apk")
            nc.sync.dma_start(out=apk[:], in_=apk_p[:])
            ATm_ji = lambda j, i: apk[:, j * DIM + i * 128 : j * DIM + (i + 1) * 128]
            fqT_j = lambda j: apk[:, 4 * DIM + j * BS : 4 * DIM + (j + 1) * BS]
            brow = [spk[:, l * H : l * H + n] for l, n in zip(range(3), [H, H, H])]
            brow.append(spk[:, 4 * H : 4 * H + DIM])
            w0row = spk[:, 3 * H : 4 * H]

            # ---- big weight DMAs: per-layer tail first, then mega chunks.
            # Megas are m-major (12 blocks of 12*128 cols + optional m-tail
            # block of 24), chunked on block boundaries so each m-group only
            # depends on its own chunk's arrival.
            Wm, Wt = {}, {}
            for nm, p, bounds, tnm, tp, tn in (
                ("W0m", W0m_p, (0, 6144, 12288, 18456), None, None, 0),
                ("W1m", W1m_p, (0, 6144, 12288, 18456), "W1t", W1t_p, H),
                ("W2m", W2m_p, (0, 6144, 12288, 18456), "W2t", W2t_p, H),
                ("W3m", W3m_p, (0, 1536, 3072, 4608, 5120, 5632, 6144), "W3t", W3t_p, DIM),
            ):
                if tnm is not None:
                    wtt = wpool.tile([2, tn], BF, tag=tnm, name=tnm)
                    nc.sync.dma_start(out=wtt[:], in_=tp[:])
                    Wt[tnm] = wtt
                wt = wpool.tile([128, bounds[-1]], BF, tag=nm, name=nm)
                for a, b in zip(bounds[:-1], bounds[1:]):
                    nc.sync.dma_start(out=wt[:, a:b], in_=p[:, a:b])
                Wm[nm] = wt

            # ---- t row: tm1row[1,64] = (t - 1) broadcast, bf16 ----
            tm1 = io.tile([1, 1], F32, tag="tm1")
            nc.vector.tensor_scalar_add(tm1[:], t_sb[:], -1.0)
            tm1row_f = io.tile([1, BS], F32, tag="tm1row_f")
            nc.vector.tensor_scalar_mul(tm1row_f[:], ones_f[:], tm1[:])
            tm1row = io.tile([1, BS], BF, tag="tm1row")
            nc.vector.tensor_copy(tm1row[:], tm1row_f[:])

            # ---- xC/xS: cos/sin of phases, feature-on-partition bf16 ----
            xC, xS = [], []
            for j in range(4):
                c = io.tile([128, BS], BF, tag=f"xC{j}", name=f"xC{j}")
                nc.scalar.activation(c[:], yT_j(j), AF.Sin, bias=pih[:])
                xC.append(c)
                s = io.tile([128, BS], BF, tag=f"xS{j}", name=f"xS{j}")
                nc.scalar.activation(s[:], yT_j(j), AF.Sin)
                xS.append(s)

            # ---- trig matmuls: AST/ACT [feature, batch], fsT combine ----
            psAS = ps.tile([128, 4 * BS], F32, tag="pbank", name="psAS")
            psAC = ps.tile([128, 4 * BS], F32, tag="pbank", name="psAC")
            for j in range(4):
                for pt, x in ((psAS, xS), (psAC, xC)):
                    for i in range(4):
                        nc.tensor.matmul(
                            pt[:, i * BS : (i + 1) * BS],
                            ATm_ji(j, i),
                            x[j][:],
                            start=(j == 0),
                            stop=(j == 3),
                        )
            fsT = []
            for i in range(4):
                f = io.tile([128, BS], F32, tag=f"fsT{i}", name=f"fsT{i}")
                nc.vector.tensor_mul(f[:], xC[i][:], psAS[:, i * BS : (i + 1) * BS])
                tmp = io.tile([128, BS], F32, tag="fstmp", bufs=2)
                nc.vector.tensor_mul(tmp[:], xS[i][:], psAC[:, i * BS : (i + 1) * BS])
                nc.vector.tensor_sub(f[:], f[:], tmp[:])
                nc.vector.tensor_scalar_mul(f[:], f[:], 1.0 / DIM)
                fsT.append(f)

            # ---- MLP layers, transposed dataflow ----
            def layer(Wm_t, Wtail_t, ktail_rhs, brow_t, rhs, mchunks, extra_k=None):
                """hT[m] = act(sum_k Wchunk[k,m]^T @ rhs[k] + b[m]).
                Wm_t is the m-major mega: block m at m*1536, m-tail block (if
                any) at 12*1536 with k-stride 2."""
                outs = []
                for moff, mw in mchunks:
                    p = ps.tile([128, BS], F32, tag="pbank", name=f"ps_{moff}")
                    pa = p[0:mw, :]
                    for k in range(12):
                        off = (
                            (moff // 128) * 1536 + k * 128
                            if mw == 128
                            else 12 * 1536 + k * 2
                        )
                        nc.tensor.matmul(
                            pa,
                            Wm_t[:, off : off + mw],
                            rhs[k],
                            start=(k == 0),
                            stop=False,
                        )
                    if Wtail_t is not None:
                        nc.tensor.matmul(
                            pa,
                            Wtail_t[:, moff : moff + mw],
                            ktail_rhs,
                            start=False,
                            stop=False,
                        )
                    if extra_k is not None:
                        xrow, xrhs = extra_k
                        nc.tensor.matmul(
                            pa,
                            xrow[:, moff : moff + mw],
                            xrhs[:],
                            start=False,
                            stop=False,
                        )
                    nc.tensor.matmul(
                        pa,
                        brow_t[:, moff : moff + mw],
                        ones_row[:],
                        start=False,
                        stop=True,
                    )
                    outs.append((p, pa, moff, mw))
                return outs

            def act_chunks(pre, fn):
                outs = []
                for p, pa, moff, mw in pre:
                    h = hp.tile([128, BS], BF, tag="h", name=f"h_{moff}")
                    nc.scalar.activation(h[0:mw, :], pa, fn)
                    outs.append(h)
                return outs

            l0_rhs = (
                [c[:] for c in xC]
                + [s[:] for s in xS]
                + [fqT_j(j) for j in range(4)]
            )
            l0 = layer(
                Wm["W0m"], None, None, brow[0],
                l0_rhs, M_CHUNKS, extra_k=(w0row, tm1row),
            )
            h = act_chunks(l0, AF.Tanh)
            for l in (1, 2):
                pre = layer(
                    Wm[f"W{l}m"], Wt[f"W{l}t"], h[12][0:2, :], brow[l],
                    [hh[:] for hh in h[:12]], M_CHUNKS,
                )
                h = act_chunks(pre, AF.Tanh)
            l3 = layer(
                Wm["W3m"], Wt["W3t"], h[12][0:2, :], brow[3],
                [hh[:] for hh in h[:12]], M3_CHUNKS,
            )

            # ---- outputs: per-chunk force DMAs (m0-m2 fly out while W3 is
            # still streaming; only m3 + f1 land in the tail) ----
            psf1 = ps.tile([1, BS], F32, tag="pbank", name="psf1")
            fmall = io.tile([128, 5 * BS], F32, tag="fmall")
            for mi, (p, pa, moff, mw) in enumerate(l3):
                fs_ = fmall[:, mi * BS : (mi + 1) * BS]
                nc.vector.tensor_mul(fs_, pa, fsT[mi][:])
                nc.vector.tensor_add(fs_, fs_, fqTf_j(mi))
                nc.sync.dma_start(
                    out=out_p[:, mi * BS : (mi + 1) * BS], in_=fs_
                )
                sq = io.tile([128, BS], BF, tag="sq", name=f"sq{mi}", bufs=4)
                nc.scalar.activation(sq[:], pa, AF.Square)
                nc.tensor.matmul(
                    psf1[:], ones128[:], sq[:], start=(mi == 0), stop=(mi == 3)
                )
            nc.vector.tensor_copy(fmall[0:1, 4 * BS : 5 * BS], psf1[:])
            nc.sync.dma_start(
                out=out_p[0:1, 4 * BS : 5 * BS], in_=fmall[0:1, 4 * BS : 5 * BS]
            )

        if loop is not None:
            with tc.For_i(0, loop, 1):
                for _rep in range(reps):
                    _emit(_rep)
        else:
            for _rep in range(reps):
                _emit(_rep)

    _split_waits(nc)
    return nc


def _prep_shared(inputs):
    """Host-side: cast weights to bf16 and pre-chunk into SBUF mega layouts.
    Weight megas are m-major: 12 blocks [12*128 cols] (block m holds cols
    m*128:(m+1)*128 of W for all 12 k-chunks), then an optional m-tail
    block [12*2 cols] for out-dim 1538."""
    f32 = lambda k: np.asarray(inputs[k], np.float32)

    def mega_mm(Wk, out_w):
        # Wk: [1536, out_w] (k rows only)
        blocks = [
            np.ascontiguousarray(Wk[:, m * 128 : (m + 1) * 128])
            .reshape(12, 128, 128)
            .transpose(1, 0, 2)
            .reshape(128, 12 * 128)
            for m in range(out_w // 128)
        ]
        if out_w % 128:
            blocks.append(
                np.ascontiguousarray(Wk[:, (out_w // 128) * 128 :])
                .reshape(12, 128, out_w % 128)
                .transpose(1, 0, 2)
                .reshape(128, 12 * (out_w % 128))
            )
        return np.ascontiguousarray(np.concatenate(blocks, axis=1)).astype(BF16)

    def mega_km(W, ncol):
        n = W.shape[0] // 128 * 128
        return np.ascontiguousarray(
            W[:n].reshape(-1, 128, ncol).transpose(1, 0, 2).reshape(128, -1)
        ).astype(BF16)

    W0 = f32("W0")
    w0x = np.concatenate([W0[0:1024], W0[1025:1537]], axis=0)  # drop t row
    A = f32("A")
    AT = np.ascontiguousarray(A.T)
    # spk row: b0 | b1 | b2 | w0row | b3
    spk = np.concatenate(
        [f32("b0"), f32("b1"), f32("b2"), W0[1024], f32("b3")]
    ).reshape(1, -1)
    shared = {
        "W0m": mega_mm(w0x, H),
        "W1m": mega_mm(f32("W1")[:1536], H),
        "W1t": np.ascontiguousarray(f32("W1")[1536:1538]).astype(BF16),
        "W2m": mega_mm(f32("W2")[:1536], H),
        "W2t": np.ascontiguousarray(f32("W2")[1536:1538]).astype(BF16),
        "W3m": mega_mm(f32("W3")[:1536], DIM),
        "W3t": np.ascontiguousarray(f32("W3")[1536:1538]).astype(BF16),
        "spk": spk.astype(BF16),
    }
    return shared, mega_km(AT, DIM), f32("t")


def _make_in_maps(inputs):
    shared, ATm, t = _prep_shared(inputs)
    y = np.asarray(inputs["y"], np.float32)
    fq = np.asarray(inputs["freqs"], np.float32)
    maps = []
    for c in range(NCORES):
        m = dict(shared)
        yT = np.ascontiguousarray(y[c * BS : (c + 1) * BS, 0:DIM].T)  # [512, 64]
        yTc = yT.reshape(4, 128, BS).transpose(1, 0, 2).reshape(128, 4 * BS)
        fqT = np.ascontiguousarray(fq[c * BS : (c + 1) * BS].T)
        fqTc = fqT.reshape(4, 128, BS).transpose(1, 0, 2).reshape(128, 4 * BS)
        ypk = np.zeros((128, 8 * BS + 1), np.float32)
        ypk[:, 0 : 4 * BS] = yTc
        ypk[:, 4 * BS : 8 * BS] = fqTc
        ypk[0, 8 * BS] = t[0]
        m["ypk"] = ypk
        apk = np.concatenate([ATm, fqTc.astype(BF16)], axis=1)
        m["apk"] = np.ascontiguousarray(apk)
        maps.append(m)
    return maps


def _assemble(out_full):
    """out_full: [NCORES, 128, 5*BS] -> [BATCH, DIM+1].
    Core layout: cols mi*BS:(mi+1)*BS hold forceT chunk mi ([feature 128,
    batch BS]); row 0 of cols 4*BS:5*BS holds f1."""
    outs = []
    for c in range(NCORES):
        O = out_full[c]
        force = (
            O[:, : 4 * BS].reshape(128, 4, BS).transpose(1, 0, 2).reshape(DIM, BS).T
        )  # [BS, DIM]
        f1 = O[0, 4 * BS : 5 * BS]  # [BS]
        outs.append(np.concatenate([force, f1[:, None]], axis=1))
    return np.ascontiguousarray(np.concatenate(outs, axis=0)).astype(np.float32)


_NC_CACHE = {}


def kernel(**inputs):
    key = "nc"
    if key not in _NC_CACHE:
        _NC_CACHE[key] = _build()
    nc = _NC_CACHE[key]

    in_maps = _make_in_maps(inputs)
    res = run_bass_kernel_spmd(nc, in_maps, core_ids=list(range(NCORES)))
    out_full = np.stack([res.results[i]["out"] for i in range(NCORES)], axis=0)
    return _assemble(out_full)
